# revision 48
# baseline (speedup 1.0000x reference)
"""Trainium2 Bass kernel for a 2-layer CIN (compressed interaction network).

Computation (see docstring math):
  x: (1024, 39, 32)
  h0 = relu(w0 @ (x outer x) + b0)   -> (B, 256, 32)
  h1a, h1b = split(h0, 2, axis=1)
  h1 = relu(w1 @ (x outer h1a) + b1) -> (B, 128, 32)
  out = sum_d concat([h1b, h1], 1)   -> (B, 256)

Strategy: data-parallel over batch on 8 cores (128 batches each). Per core,
activations live as (channel, B*D) with B*D = 4096 columns processed in 8
chunks of 512 (one PSUM bank). The outer-product features Z[(m,c), col] =
X[m,col]*H[c,col] are formed by (a) replicating row m across partitions with
a one-hot fp32r matmul on the tensor engine, then (b) an elementwise multiply
on the vector/gpsimd engines. The 1x1-conv GEMMs contract Z against
pre-transposed weights in fp32r (4x the fp32 matmul rate; ~1.6e-4 rel err).
"""

import numpy as np

import concourse.bacc as bacc
import concourse.mybir as mybir
import concourse.tile as tile
from concourse.bass_utils import run_bass_kernel_spmd

F32 = mybir.dt.float32
F32R = mybir.dt.float32r
AL = mybir.AluOpType
AX = mybir.AxisListType
ACTF = mybir.ActivationFunctionType

N_CORES = 8
B_FULL, M, D = 1024, 39, 32
L0_OUT, L1_OUT = 256, 128
B_LOC = B_FULL // N_CORES          # 128 batches per core
BD = B_LOC * D                     # 4096 columns per core
CHUNK = 512                        # columns per chunk (one PSUM bank, fp32)
B_CHUNK = CHUNK // D               # 16 batches per chunk
N_CHUNKS = BD // CHUNK             # 8
# Symmetric layer 0: only upper-triangular (m, n) pairs with folded weights.
_PAIRS = [(m, n) for m in range(M) for n in range(m, M)]   # 780
NT0 = (len(_PAIRS) + 127) // 128                           # 7 K-tiles
K0PAD = NT0 * 128                                          # 896

# L1 multiply engine split per m:
#   'a' -> Pool, broadcast via gpsimd partition_broadcast (no PE/ACT involved)
#   'b' -> Pool, broadcast via PE one-hot matmul + ACT copy to SBUF
#   'd' -> DVE, broadcast via PE one-hot matmul (reads PSUM directly)
_POOL_A = set()
_POOL_B = {m for m in range(M) if m % 3 == 0}
_L1_ENG = ["a" if m in _POOL_A else "b" if m in _POOL_B else "d"
           for m in range(M)]


def _build_nc(repeat=1):
    nc = bacc.Bacc("TRN2", target_bir_lowering=False)

    x = nc.dram_tensor("x", [B_LOC, M, D], F32R, kind="ExternalInput")
    w0t = nc.dram_tensor("w0t", [K0PAD, L0_OUT], F32R, kind="ExternalInput")
    w1t = nc.dram_tensor("w1t", [M * L1_OUT, L1_OUT], F32R, kind="ExternalInput")
    b0 = nc.dram_tensor("b0", [L0_OUT, 1], F32, kind="ExternalInput")
    b1 = nc.dram_tensor("b1", [L1_OUT, 1], F32, kind="ExternalInput")
    ohl = nc.dram_tensor("ohl", [M, NT0, 128], F32R, kind="ExternalInput")
    ohr = nc.dram_tensor("ohr", [M, NT0, 128], F32R, kind="ExternalInput")
    oh1 = nc.dram_tensor("oh1", [M, M, L1_OUT], F32R, kind="ExternalInput")
    out = nc.dram_tensor("out", [L0_OUT, B_LOC], F32, kind="ExternalOutput")

    with tile.TileContext(nc) as tc:
        with (
            tc.tile_pool(name="const", bufs=1) as const,
            tc.tile_pool(name="sb", bufs=4) as sb,
            tc.tile_pool(name="s0pool", bufs=NT0 + 1) as s0pool,
            tc.tile_pool(name="s1pool", bufs=28) as s1pool,
            tc.tile_pool(name="bcp", bufs=4, space="PSUM") as bcp,
            tc.tile_pool(name="bcr", bufs=1, space="PSUM") as bcr,
            tc.tile_pool(name="acc", bufs=1, space="PSUM") as acc,
            # bcp: bcA (L0 left / L1 bcast) x3 + bcr: bcR x2 = 5 banks
            # acc: h0a, h0b, h1 = 3 banks  -> 8 total
        ):
            # ---- persistent loads, ordered so chunk-0 layer-0 deps land first
            xa = const.tile([M, B_LOC, D], F32R)          # X[m, (b d)]
            xr = x.rearrange("b m d -> m b d")
            bs0 = slice(0, B_CHUNK)
            nc.sync.dma_start(xa[:, bs0, :], xr[:, bs0, :])

            ohl_sb = const.tile([M, NT0, 128], F32R)
            nc.sync.dma_start(ohl_sb[:], ohl[:])
            ohr_sb = const.tile([M, NT0, 128], F32R)
            nc.sync.dma_start(ohr_sb[:], ohr[:])
            w0t_sb = const.tile([128, NT0, L0_OUT], F32R)
            w0tr = w0t.rearrange("(t p) o -> p t o", p=128)
            for t in range(NT0):
                nc.sync.dma_start(w0t_sb[:, t, :], w0tr[:, t, :])

            b0a_sb = const.tile([128, 1], F32)
            nc.sync.dma_start(b0a_sb[:], b0[0:128, :])
            b0b_sb = const.tile([128, 1], F32)
            nc.sync.dma_start(b0b_sb[:], b0[128:256, :])
            b1_sb = const.tile([128, 1], F32)
            nc.sync.dma_start(b1_sb[:], b1[:])

            # needed from layer 1 of chunk 0 onward — stream under L0 compute
            oh1_sb = const.tile([M, M, L1_OUT], F32R)
            for m0 in range(0, M, 5):
                ms = slice(m0, min(m0 + 5, M))
                nc.sync.dma_start(oh1_sb[:, ms, :], oh1[:, ms, :])
            w1t_sb = const.tile([L1_OUT, M, L1_OUT], F32R)
            w1tr = w1t.rearrange("(m p) o -> p m o", p=L1_OUT)
            for m0 in range(0, M, 3):
                ms = slice(m0, min(m0 + 3, M))
                nc.sync.dma_start(w1t_sb[:, ms, :], w1tr[:, ms, :])
            for n in range(1, N_CHUNKS):
                bs = slice(n * B_CHUNK, (n + 1) * B_CHUNK)
                nc.sync.dma_start(xa[:, bs, :], xr[:, bs, :])

            r0 = const.tile([128, B_LOC], F32)   # sum_d relu(h0b): out rows 0:128
            r1 = const.tile([128, B_LOC], F32)   # sum_d relu(h1):  out rows 128:256

            for n in range(N_CHUNKS * repeat):
                n = n % N_CHUNKS
                bs = slice(n * B_CHUNK, (n + 1) * B_CHUNK)
                xa_n = xa[:, bs, :].rearrange("m b d -> m (b d)")

                # ---- layer 0 (symmetric upper-tri Z with folded weights) ----
                # produce all s-tiles first, then run the GEMM chain
                s_tiles = []
                for t in range(NT0):
                    l_ps = bcp.tile([128, CHUNK], F32, tag="bcA")
                    nc.tensor.matmul(l_ps[:], ohl_sb[:, t, :], xa_n,
                                     start=True, stop=True)
                    r_ps = bcr.tile([128, CHUNK], F32, tag="bcR")
                    nc.tensor.matmul(r_ps[:], ohr_sb[:, t, :], xa_n,
                                     start=True, stop=True)
                    r_sb = sb.tile([128, CHUNK], F32, tag="rsb")
                    nc.scalar.copy(r_sb[:], r_ps[:])
                    s = s0pool.tile([128, CHUNK], F32R, tag="s0")
                    nc.vector.tensor_tensor(s[:], r_sb[:], l_ps[:], AL.mult)
                    s_tiles.append(s)
                h0a_ps = acc.tile([128, CHUNK], F32, tag="h0a")
                h0b_ps = acc.tile([128, CHUNK], F32, tag="h0b")
                for t, s in enumerate(s_tiles):
                    nc.tensor.matmul(h0a_ps[:], w0t_sb[:, t, 0:128], s[:],
                                     start=(t == 0), stop=(t == NT0 - 1))
                    nc.tensor.matmul(h0b_ps[:], w0t_sb[:, t, 128:256], s[:],
                                     start=(t == 0), stop=(t == NT0 - 1))

                h1a_sb = sb.tile([128, CHUNK], F32, tag="h1a")
                nc.scalar.activation(h1a_sb[:], h0a_ps[:], ACTF.Relu,
                                     bias=b0a_sb[:], scale=1.0)
                h0b_sb = sb.tile([128, B_CHUNK, D], F32, tag="h0b")
                nc.scalar.activation(
                    h0b_sb[:].rearrange("p b d -> p (b d)"), h0b_ps[:],
                    ACTF.Relu, bias=b0b_sb[:], scale=1.0)
                nc.vector.tensor_reduce(r0[:, bs], h0b_sb[:], AX.X, AL.add)
                if n == N_CHUNKS - 1:
                    nc.sync.dma_start(out[0:128, :], r0[:])

                # ---- layer 1 ----
                # s2 production interleaved with the GEMM chain (same order)
                m_order = list(range(M))
                s2_tiles = {}
                for m in m_order:
                    s2 = s1pool.tile([128, CHUNK], F32R, tag="s1")
                    if _L1_ENG[m] == "a":
                        bc_sb = sb.tile([128, CHUNK], F32, tag="bcsb")
                        nc.gpsimd.partition_broadcast(
                            bc_sb[:], xa_n[m:m + 1, :].bitcast(F32))
                        nc.gpsimd.tensor_tensor(s2[:], h1a_sb[:], bc_sb[:], AL.mult)
                    elif _L1_ENG[m] == "b":
                        bc = bcp.tile([128, CHUNK], F32, tag="bcA")
                        nc.tensor.matmul(bc[:], oh1_sb[:, m, :], xa_n,
                                         start=True, stop=True)
                        bc_sb = sb.tile([128, CHUNK], F32, tag="bcsb")
                        nc.scalar.copy(bc_sb[:], bc[:])
                        nc.gpsimd.tensor_tensor(s2[:], h1a_sb[:], bc_sb[:], AL.mult)
                    else:
                        bc = bcp.tile([128, CHUNK], F32, tag="bcA")
                        nc.tensor.matmul(bc[:], oh1_sb[:, m, :], xa_n,
                                         start=True, stop=True)
                        nc.vector.tensor_tensor(s2[:], h1a_sb[:], bc[:], AL.mult)
                    s2_tiles[m] = s2
                h1_ps = acc.tile([128, CHUNK], F32, tag="h1")
                for i, m in enumerate(m_order):
                    nc.tensor.matmul(h1_ps[:], w1t_sb[:, m, :], s2_tiles[m][:],
                                     start=(i == 0), stop=(i == M - 1))

                h1_sb = sb.tile([128, B_CHUNK, D], F32, tag="h1sb")
                nc.scalar.activation(
                    h1_sb[:].rearrange("p b d -> p (b d)"), h1_ps[:],
                    ACTF.Relu, bias=b1_sb[:], scale=1.0)
                nc.vector.tensor_reduce(r1[:, bs], h1_sb[:], AX.X, AL.add)
                if n == N_CHUNKS - 1:
                    nc.sync.dma_start(out[128:256, :], r1[:])

    nc.finalize()
    return nc


def _host_inputs(x, w0, b0, w1, b1):
    """Per-core input dicts (host-side layout prep only)."""
    x = np.ascontiguousarray(x, dtype=np.float32)
    w1t = np.ascontiguousarray(w1.T, dtype=np.float32)
    b0c = np.ascontiguousarray(b0.reshape(L0_OUT, 1), dtype=np.float32)
    b1c = np.ascontiguousarray(b1.reshape(L1_OUT, 1), dtype=np.float32)

    # folded symmetric layer-0 weights over upper-tri pairs
    w0r3 = np.asarray(w0, dtype=np.float64).reshape(L0_OUT, M, M)
    wsym = w0r3 + w0r3.transpose(0, 2, 1)
    idx = np.arange(M)
    wsym[:, idx, idx] = w0r3[:, idx, idx]
    w0t = np.zeros((K0PAD, L0_OUT), dtype=np.float32)
    ohl = np.zeros((M, NT0, 128), dtype=np.float32)
    ohr = np.zeros((M, NT0, 128), dtype=np.float32)
    for r, (m, n) in enumerate(_PAIRS):
        t, p = divmod(r, 128)
        ohl[m, t, p] = 1.0
        ohr[n, t, p] = 1.0
        w0t[r, :] = wsym[:, m, n].astype(np.float32)
    oh1 = np.zeros((M, M, L1_OUT), dtype=np.float32)
    for m in range(M):
        oh1[m, m, :] = 1.0

    shared = {"w0t": w0t, "w1t": w1t, "b0": b0c, "b1": b1c,
              "ohl": ohl, "ohr": ohr, "oh1": oh1}
    in_maps = []
    for c in range(N_CORES):
        im = dict(shared)
        im["x"] = np.ascontiguousarray(x[c * B_LOC:(c + 1) * B_LOC])
        in_maps.append(im)
    return in_maps


_NC_CACHE = {}


def _get_nc(repeat=1):
    if repeat not in _NC_CACHE:
        _NC_CACHE[repeat] = _build_nc(repeat)
    return _NC_CACHE[repeat]


def _run(x, w0, b0, w1, b1, **run_kwargs):
    in_maps = _host_inputs(x, w0, b0, w1, b1)
    res = run_bass_kernel_spmd(_get_nc(), in_maps,
                               core_ids=list(range(N_CORES)), **run_kwargs)
    parts = [res.results[c]["out"].T for c in range(N_CORES)]  # (128, 256) each
    return np.concatenate(parts, axis=0).astype(np.float32), res


def kernel(x, w0, b0, w1, b1):
    out, _ = _run(x, w0, b0, w1, b1)
    return out
